# revision 1
# baseline (speedup 1.0000x reference)
"""DeformableAttention1D on 8 TRN2 NeuronCores via Bass/Tile.

Sharding: core c handles offset-group g=c//2 (64 of 256 channels, 2 of 8 heads)
and query-half qh=c%2 (512 of 1024 positions). Each core computes its group's
offsets/gather/CPB/attention independently; the final output projection is
computed as a partial (wo sliced by group) and summed on the host (the
"all-reduce" of the output projection).

Device-side numerics: fp32 everywhere except the CPB relative-position-bias
MLP and its broadcast, which use fp32r matmuls (1 cycle/column vs 4 for fp32).
The ACT engine is restricted to ONE table set (natural_log_exp_and_others:
Exp/Ln/Relu/Copy/Identity/Square) because runtime table swaps are broken in
this environment; tanh and erf(gelu) are composed from Exp + DVE ops.
"""
import os
import sys

sys.path.insert(0, "/opt/trn_rl_repo")

DEBUG = bool(os.environ.get("DEFORM_DEBUG"))

import numpy as np

import concourse.bacc as bacc
import concourse.bass as bass
import concourse.mybir as mybir
import concourse.tile as tile
import concourse.bass_utils as bass_utils

F32 = mybir.dt.float32
F32R = mybir.dt.float32r
I32 = mybir.dt.int32
U32 = mybir.dt.uint32
AF = mybir.ActivationFunctionType
ALU = mybir.AluOpType

# model dims (hardcoded per problem spec)
DIM = 256
N = 1024
G = 4
HEADS = 8
DH = 32
NDS = 256          # downsampled kv positions
QS = 512           # queries per core
DPG = 64           # channels per group
OFF_K = 6
DS = 4             # downsample stride
OFF_SCALE = 4.0
NCORES = 8

# A&S 7.1.26 erf coefficients (|err| <= 1.5e-7)
ERF_P = 0.3275911
ERF_A = [0.254829592, -0.284496736, 1.421413741, -1.453152027, 1.061405429]

_CACHED = {}


def _patch_act_tables():
    """Restrict activation-table selection to the single set that covers all
    ACT functions used by this kernel, so exactly one table load is emitted
    (runtime table swaps do not work in this environment)."""
    import concourse.hw_specs as hw_specs

    if getattr(bacc, "_deform_act_patch", False):
        return
    orig = hw_specs.get_activation_tables

    keep = "natural_log_exp_and_others"

    def patched(module_arch):
        tabs = orig(module_arch)
        keep_funcs = tabs[keep]
        out = {}
        for name, funcs in tabs.items():
            if name == keep:
                out[name] = funcs
            else:
                out[name] = funcs - keep_funcs
        return out

    bacc.get_activation_tables = patched
    bacc._deform_act_patch = True


def _erf_gelu(nc, sb, out_ap, x_ap, shape):
    """out = 0.5 * x * (1 + erf(x/sqrt(2))) via A&S 7.1.26 (no erf table).

    Writes (1 + erf(x/sqrt2)) * x  (WITHOUT the 0.5 -- folded into wproj).
    """
    P, Nf = shape
    sq = sb.tile([P, Nf], F32, name="gelu_sq", tag="gelu_sq")
    nc.scalar.activation(sq[:], x_ap, AF.Square)
    e = sb.tile([P, Nf], F32, name="gelu_e", tag="gelu_e")
    # e = exp(-x^2/2)
    nc.scalar.activation(e[:], sq[:], AF.Exp, scale=-0.5)
    ax = sb.tile([P, Nf], F32, name="gelu_ax", tag="gelu_ax")
    # |x|/sqrt(2) = max(x, -x) * (1/sqrt2): two steps
    nc.vector.scalar_tensor_tensor(ax[:], x_ap, -1.0, x_ap, ALU.mult, ALU.max)
    t = sb.tile([P, Nf], F32, name="gelu_t", tag="gelu_t")
    # t = 1 / (1 + p * |x| / sqrt2)
    nc.vector.tensor_scalar(t[:], ax[:], float(ERF_P / np.sqrt(2.0)), 1.0, ALU.mult, ALU.add)
    nc.vector.reciprocal(t[:], t[:])
    poly = sb.tile([P, Nf], F32, name="gelu_poly", tag="gelu_poly")
    # P(t) = a1 t + a2 t^2 + ... + a5 t^5 via (x + c)*t nested form
    nc.vector.tensor_scalar(poly[:], t[:], ERF_A[4], ERF_A[3], ALU.mult, ALU.add)
    nc.vector.tensor_tensor(poly[:], poly[:], t[:], ALU.mult)
    nc.vector.scalar_tensor_tensor(poly[:], poly[:], ERF_A[2], t[:], ALU.add, ALU.mult)
    nc.vector.scalar_tensor_tensor(poly[:], poly[:], ERF_A[1], t[:], ALU.add, ALU.mult)
    nc.vector.scalar_tensor_tensor(poly[:], poly[:], ERF_A[0], t[:], ALU.add, ALU.mult)
    # poly*e = 1 - erf(|x|/sqrt2)  =>  erfa = 1 - poly*e
    erfa = sb.tile([P, Nf], F32, name="gelu_erfa", tag="gelu_erfa")
    nc.vector.tensor_tensor(erfa[:], poly[:], e[:], ALU.mult)
    nc.vector.tensor_scalar(erfa[:], erfa[:], -1.0, 1.0, ALU.mult, ALU.add)
    # copysign: erf(x) = sign(x)*erfa
    sgn = sb.tile([P, Nf], U32, name="gelu_sgn", tag="gelu_sgn")
    nc.vector.tensor_scalar(sgn[:], x_ap.bitcast(U32), 0x80000000, None, ALU.bitwise_and)
    erfs = sb.tile([P, Nf], F32, name="gelu_erfs", tag="gelu_erfs")
    nc.vector.tensor_tensor(erfs[:].bitcast(U32), erfa[:].bitcast(U32), sgn[:], ALU.bitwise_or)
    # out = (1 + erf) * x    (0.5 folded into wproj)
    nc.vector.tensor_scalar(erfs[:], erfs[:], 1.0, None, ALU.add)
    nc.vector.tensor_tensor(out_ap, erfs[:], x_ap, ALU.mult)


def _tanh_rows(nc, sb, out_ap, x_ap, shape):
    """out = tanh(x) = sign(x) * (1 - 2/(exp(2*min(|x|,30))+1)) on small tiles."""
    P, Nf = shape
    ax = sb.tile([P, Nf], F32, name="th_ax", tag="th_ax")
    nc.vector.scalar_tensor_tensor(ax[:], x_ap, -1.0, x_ap, ALU.mult, ALU.max)
    nc.vector.tensor_scalar(ax[:], ax[:], 30.0, None, ALU.min)
    e = sb.tile([P, Nf], F32, name="th_e", tag="th_e")
    nc.scalar.activation(e[:], ax[:], AF.Exp, scale=2.0)
    nc.vector.tensor_scalar(e[:], e[:], 1.0, None, ALU.add)
    r = sb.tile([P, Nf], F32, name="th_r", tag="th_r")
    nc.vector.reciprocal(r[:], e[:])
    # tha = 1 - 2r
    nc.vector.tensor_scalar(r[:], r[:], -2.0, 1.0, ALU.mult, ALU.add)
    sgn = sb.tile([P, Nf], U32, name="th_sgn", tag="th_sgn")
    nc.vector.tensor_scalar(sgn[:], x_ap.bitcast(U32), 0x80000000, None, ALU.bitwise_and)
    nc.vector.tensor_tensor(out_ap.bitcast(U32), r[:].bitcast(U32), sgn[:], ALU.bitwise_or)


def build_nc():
    _patch_act_tables()
    nc = bacc.Bacc("TRN2", target_bir_lowering=False, debug=False, num_devices=NCORES)

    # ---- per-core DRAM inputs ----
    din = {}

    def dt_in(name, shape):
        din[name] = nc.dram_tensor(name, shape, F32, kind="ExternalInput")
        return din[name]

    dt_in("xg", [DPG, N])
    dt_in("xq", [DPG, QS])
    dt_in("mask_st", [128, 32 * 128])
    # all small weights + identity packed into one tensor (one DMA)
    dt_in("packed", [128, 788])
    y_out = nc.dram_tensor("y", [DIM, QS], F32, kind="ExternalOutput")
    dbg = {}
    if DEBUG:
        for nm, shp in [("dbg_q", [DPG, N]), ("dbg_vgsp1", [1, NDS]),
                        ("dbg_rows4", [1, 4 * NDS]), ("dbg_kv", [DPG, NDS]),
                        ("dbg_k", [DPG, NDS]), ("dbg_v", [DPG, NDS]),
                        ("dbg_qs", [DPG, QS]), ("dbg_t0", [128, QS]),
                        ("dbg_bstk0", [128, NDS]), ("dbg_logit00", [128, QS]),
                        ("dbg_avn", [DPG, QS])]:
            dbg[nm] = nc.dram_tensor(nm, shp, F32, kind="ExternalOutput")

    NT = N // 128          # 8 n-tiles for gather
    NITER = QS // 2        # 256 CPB iterations (2 queries each)
    NSTACK = NITER // 32   # 8 bias stacks

    with tile.TileContext(nc) as tc:
        with (
            tc.tile_pool(name="const", bufs=1) as cst,
            tc.tile_pool(name="work", bufs=2) as wk,
            tc.tile_pool(name="rows", bufs=1) as rw,
            tc.tile_pool(name="persist", bufs=1) as pe_pool,
            tc.tile_pool(name="h1p", bufs=4) as h1p,
            tc.tile_pool(name="h2p", bufs=4) as h2p,
        ):
            # ---- load inputs: xg, xq, then one packed-weights DMA ----
            xg = cst.tile([DPG, N], F32, name="xg", tag="xg")
            nc.sync.dma_start(xg[:], din["xg"].ap())
            xq = cst.tile([DPG, QS], F32, name="xq", tag="xq")
            nc.sync.dma_start(xq[:], din["xq"].ap())
            packed = cst.tile([128, 788], F32, name="packed", tag="packed")
            nc.sync.dma_start(packed[:], din["packed"].ap())
            w2bd = packed[:, 0:128]
            eyet = packed[:, 128:256]
            wqT = packed[0:DPG, 256:320]
            wqTs = packed[0:DPG, 320:384]
            wkT = packed[0:DPG, 384:448]
            wvT = packed[0:DPG, 448:512]
            woT = packed[0:DPG, 512:768]
            wdw = packed[0:DPG, 768:774]
            bodw = packed[0:DPG, 774:775]
            wproj_half = packed[0:DPG, 775:776]
            b1col = packed[:, 776:777]
            b2col = packed[:, 777:778]
            b3bc = packed[:, 778:780]
            qbase = packed[:, 780:781]
            w3bd = packed[:, 781:785]

            ones_col = cst.tile([128, 1], F32, name="ones", tag="ones")
            nc.gpsimd.memset(ones_col[:], 1.0)
            # dummy activation: triggers the (single) ACT table load at t=0 so
            # it overlaps the input DMAs instead of sitting in the offsets chain
            warm = cst.tile([128, 1], F32, name="warm", tag="warm")
            nc.scalar.activation(warm[:], ones_col[:], AF.Relu)
            ones_colr = cst.tile([128, 1], F32R, name="onesr", tag="onesr")
            nc.vector.tensor_copy(ones_colr[:], ones_col[:])

            # fp32r copies of CPB weights
            w2bdr = cst.tile([128, 128], F32R, name="w2bdr", tag="w2bdr")
            nc.vector.tensor_copy(w2bdr[:], w2bd)
            w3bdr = cst.tile([128, 4], F32R, name="w3bdr", tag="w3bdr")
            nc.vector.tensor_copy(w3bdr[:], w3bd)
            woTr = cst.tile([DPG, DIM], F32R, name="woTr", tag="woTr")
            nc.vector.tensor_copy(woTr[:], woT)


            # persistent SBUF tiles that cross phase boundaries
            k_sb = pe_pool.tile([DPG, NDS], F32R, name="k_sb", tag="k_sb")
            qs_sb = pe_pool.tile([DPG, QS], F32R, name="qs_sb", tag="qs_sb")
            vT = [pe_pool.tile([128, DPG], F32R, name=f"vT{H}", tag=f"vT{H}") for H in range(2)]
            tT = [pe_pool.tile([128, NDS], F32R, name=f"tT{t}", tag=f"tT{t}") for t in range(4)]
            # bias, transposed into attention layout, packed as
            # col = ((((itq*2 + itl)*32 + pp)*2 + h1)*2 + c)*2 + o  (j on partitions)
            biasT_sb = pe_pool.tile([128, 4 * QS], F32, name="biasT_sb", tag="biasT_sb")
            avn = pe_pool.tile([DPG, QS], F32R, name="avn", tag="avn")

            # ============ phases A-C: q, offsets, gather, kv, t ============
            with tc.tile_pool(name="psA", bufs=2, space="PSUM") as psA:
                # ---- phase A: q + offsets ----
                q_pad = pe_pool.tile([DPG, N + 2], F32, name="q_pad", tag="q_pad")
                nc.gpsimd.memset(q_pad[:], 0.0)
                for h in range(2):
                    pq = psA.tile([DPG, QS], F32, name="pA512", tag="pA512")
                    nc.tensor.matmul(pq[:], wqT, xg[:, h * QS:(h + 1) * QS])
                    nc.scalar.copy(q_pad[:, 1 + h * QS:1 + (h + 1) * QS], pq[:])

                # depthwise strided conv (6 taps)
                acc = wk.tile([DPG, NDS], F32, name="conv_acc", tag="conv_acc")
                nc.vector.tensor_scalar(
                    acc[:], q_pad[:, 0:N - 3:DS], wdw[:, 0:1], bodw, ALU.mult, ALU.add)
                for kk in range(1, OFF_K):
                    nc.vector.scalar_tensor_tensor(
                        acc[:], q_pad[:, kk:kk + N - 3:DS], wdw[:, kk:kk + 1], acc[:],
                        ALU.mult, ALU.add)

                if DEBUG:
                    nc.sync.dma_start(dbg["dbg_q"].ap(), q_pad[:, 1:N + 1])
                gl = wk.tile([DPG, NDS], F32, name="gelu_out", tag="gelu_out")
                _erf_gelu(nc, wk, gl[:], acc[:], [DPG, NDS])

                # proj row: [1, NDS] = sum_c 0.5*wproj[c] * gl[c, :]
                pproj = psA.tile([1, NDS], F32, name="pproj", tag="pproj")
                nc.tensor.matmul(pproj[:], wproj_half, gl[:])
                proj_sb = rw.tile([1, NDS], F32, name="proj_sb", tag="proj_sb")
                nc.vector.tensor_copy(proj_sb[:], pproj[:])
                th = rw.tile([1, NDS], F32, name="th", tag="th")
                _tanh_rows(nc, rw, th[:], proj_sb[:], [1, NDS])

                # vgrid = j + 4*tanh ; vgsp1 = vgrid*2/255 ; p_pix = vgsp1*512 - 0.5
                iotaj = rw.tile([1, NDS], I32, name="iotaj", tag="iotaj")
                nc.gpsimd.iota(iotaj[:], pattern=[[1, NDS]], base=0, channel_multiplier=0)
                iotajf = rw.tile([1, NDS], F32, name="iotajf", tag="iotajf")
                nc.vector.tensor_copy(iotajf[:], iotaj[:])
                vgrid = rw.tile([1, NDS], F32, name="vgrid", tag="vgrid")
                nc.vector.scalar_tensor_tensor(vgrid[:], th[:], OFF_SCALE, iotajf[:], ALU.mult, ALU.add)
                vgsp1 = rw.tile([1, NDS], F32, name="vgsp1", tag="vgsp1")
                nc.vector.tensor_scalar(vgsp1[:], vgrid[:], float(2.0 / (NDS - 1)), None, ALU.mult)
                ppix = rw.tile([1, NDS], F32, name="ppix", tag="ppix")
                nc.vector.tensor_scalar(ppix[:], vgsp1[:], float(N / 2.0), -0.5, ALU.mult, ALU.add)

                # rows4 = [i0f | i1f | w0 | w1]
                rows4 = rw.tile([1, 4 * NDS], F32, name="rows4", tag="rows4")
                i0i = rw.tile([1, NDS], I32, name="i0i", tag="i0i")
                nc.vector.tensor_copy(i0i[:], ppix[:])
                i0c = rw.tile([1, NDS], F32, name="i0c", tag="i0c")
                nc.vector.tensor_copy(i0c[:], i0i[:])
                # floor(p) regardless of the convert rounding mode:
                # i0 = cvt(p) - (cvt(p) > p)
                gt = rw.tile([1, NDS], F32, name="gt", tag="gt")
                nc.vector.tensor_tensor(gt[:], i0c[:], ppix[:], ALU.is_gt)
                nc.vector.tensor_tensor(rows4[:, 0:NDS], i0c[:], gt[:], ALU.subtract)
                nc.vector.tensor_scalar(rows4[:, NDS:2 * NDS], rows4[:, 0:NDS], 1.0, None, ALU.add)
                nc.vector.tensor_tensor(rows4[:, 3 * NDS:4 * NDS], ppix[:], rows4[:, 0:NDS], ALU.subtract)
                nc.vector.tensor_scalar(rows4[:, 2 * NDS:3 * NDS], rows4[:, 3 * NDS:4 * NDS], -1.0, 1.0, ALU.mult, ALU.add)

                if DEBUG:
                    nc.sync.dma_start(dbg["dbg_vgsp1"].ap(), vgsp1[:])
                    nc.sync.dma_start(dbg["dbg_rows4"].ap(), rows4[:])
                bc4 = pe_pool.tile([128, 4 * NDS], F32, name="bc4", tag="bc4")
                nc.gpsimd.partition_broadcast(bc4[:], rows4[:])

                # vgsp1 as per-partition columns for the two j-halves
                # (PE transpose of the row -- avoids DMA queue latency)
                vgsp1c = cst.tile([128, 2], F32, name="vgsp1c", tag="vgsp1c")
                for H in range(2):
                    ptv = psA.tile([128, 128], F32, name="ptv", tag="ptp")
                    nc.tensor.transpose(ptv[:, 0:1], vgsp1[:, H * 128:(H + 1) * 128],
                                        eyet[0:1, 0:1])
                    nc.vector.tensor_copy(vgsp1c[:, H:H + 1], ptv[:, 0:1])

                # CPB selection masks (one prepacked DMA + fp32r round)
                maskr = pe_pool.tile([128, 32 * 128], F32R, name="maskr", tag="maskr")
                with tc.tile_pool(name="maskst", bufs=1) as mp:
                    mask_st = mp.tile([128, 32 * 128], F32, name="mask_st", tag="mask_st")
                    nc.sync.dma_start(mask_st[:], din["mask_st"].ap())
                    nc.vector.tensor_copy(maskr[:], mask_st[:])

                # ---- phase C: t = sign(pos)*log1p(|pos|), transposed ----
                io = wk.tile([128, QS], I32, name="io", tag="io")
                nc.gpsimd.iota(io[:], pattern=[[1, QS]], base=0, channel_multiplier=0)
                gqp = wk.tile([128, QS], F32, name="gqp", tag="gqp")
                nc.vector.tensor_scalar(gqp[:], io[:], qbase, float(2.0 / (N - 1)), ALU.add, ALU.mult)

                for H in range(2):
                    pos = wk.tile([128, QS], F32, name="pos", tag="pos")
                    nc.vector.tensor_scalar(pos[:], gqp[:], vgsp1c[:, H:H + 1], None, ALU.subtract)
                    apos = wk.tile([128, QS], F32, name="apos", tag="apos")
                    nc.vector.scalar_tensor_tensor(apos[:], pos[:], -1.0, pos[:], ALU.mult, ALU.max)
                    aln = wk.tile([128, QS], F32, name="aln", tag="aln")
                    nc.scalar.activation(aln[:], apos[:], AF.Ln, bias=1.0)
                    sgn = wk.tile([128, QS], U32, name="psgn", tag="psgn")
                    nc.vector.tensor_scalar(sgn[:], pos[:].bitcast(U32), 0x80000000, None, ALU.bitwise_and)
                    t_H = wk.tile([128, QS], F32, name="t_H", tag="t_H")
                    nc.vector.tensor_tensor(t_H[:].bitcast(U32), aln[:].bitcast(U32), sgn[:], ALU.bitwise_or)
                    if DEBUG and H == 0:
                        nc.sync.dma_start(dbg["dbg_t0"].ap(), t_H[:])
                    for it in range(4):
                        ptp = psA.tile([128, 128], F32, name="ptp", tag="ptp")
                        nc.tensor.transpose(ptp[:], t_H[:, it * 128:(it + 1) * 128], eyet)
                        nc.scalar.copy(tT[it][:, H * 128:(H + 1) * 128], ptp[:])

                # selection masks for the CPB broadcast. Iteration p reads
                # tT rows (2p, 2p+1); those sit inside the 32-aligned window
                # [32*(p//16), +32), so a [32, 128] mask indexed by p%16
                # suffices (16 variants).

                # ---- phase B: gather (one-hot matmul), kv, k, v, vT ----
                xgT = []
                for t in range(NT):
                    pt = psA.tile([128, 128], F32, name="ptp", tag="ptp")
                    nc.tensor.transpose(pt[:, 0:DPG], xg[:, t * 128:(t + 1) * 128], eyet[0:DPG, 0:DPG])
                    st = pe_pool.tile([128, DPG], F32, name=f"xgT{t}", tag=f"xgT{t}")
                    nc.scalar.copy(st[:], pt[:, 0:DPG])
                    xgT.append(st)

                pkv = psA.tile([DPG, NDS], F32, name="pA256", tag="pA256")
                for t in range(NT):
                    icol = wk.tile([128, 1], I32, name="icol", tag="icol")
                    nc.gpsimd.iota(icol[:], pattern=[[0, 1]], base=t * 128, channel_multiplier=1)
                    icolf = wk.tile([128, 1], F32, name="icolf", tag="icolf")
                    nc.vector.tensor_copy(icolf[:], icol[:])
                    eq0 = wk.tile([128, NDS], F32, name="eq0", tag="eq0")
                    nc.vector.tensor_scalar(eq0[:], bc4[:, 0:NDS], icolf[:], None, ALU.is_equal)
                    s0 = wk.tile([128, NDS], F32, name="s0", tag="s0")
                    nc.vector.tensor_tensor(s0[:], eq0[:], bc4[:, 2 * NDS:3 * NDS], ALU.mult)
                    eq1 = wk.tile([128, NDS], F32, name="eq1", tag="eq1")
                    nc.vector.tensor_scalar(eq1[:], bc4[:, NDS:2 * NDS], icolf[:], None, ALU.is_equal)
                    s1 = wk.tile([128, NDS], F32, name="s1", tag="s1")
                    nc.vector.tensor_tensor(s1[:], eq1[:], bc4[:, 3 * NDS:4 * NDS], ALU.mult)
                    S = wk.tile([128, NDS], F32, name="S", tag="S")
                    nc.vector.tensor_tensor(S[:], s0[:], s1[:], ALU.add)
                    nc.tensor.matmul(pkv[:], xgT[t][:], S[:],
                                     start=(t == 0), stop=(t == NT - 1))
                kv = wk.tile([DPG, NDS], F32, name="kv", tag="kv")
                nc.scalar.copy(kv[:], pkv[:])
                if DEBUG:
                    nc.sync.dma_start(dbg["dbg_kv"].ap(), kv[:])

                pk = psA.tile([DPG, NDS], F32, name="pA256", tag="pA256")
                nc.tensor.matmul(pk[:], wkT, kv[:])
                nc.scalar.copy(k_sb[:], pk[:])
                pv = psA.tile([DPG, NDS], F32, name="pA256", tag="pA256")
                nc.tensor.matmul(pv[:], wvT, kv[:])
                v_sb = wk.tile([DPG, NDS], F32, name="v_sb", tag="v_sb")
                nc.scalar.copy(v_sb[:], pv[:])
                if DEBUG:
                    nc.sync.dma_start(dbg["dbg_k"].ap(), k_sb[:])
                    nc.sync.dma_start(dbg["dbg_v"].ap(), v_sb[:])

                for H in range(2):
                    pt = psA.tile([128, 128], F32, name="ptp", tag="ptp")
                    nc.tensor.transpose(pt[:, 0:DPG], v_sb[:, H * 128:(H + 1) * 128], eyet[0:DPG, 0:DPG])
                    nc.scalar.copy(vT[H][:], pt[:, 0:DPG])

                # q_s (scaled q for this core's query half)
                pqs = psA.tile([DPG, QS], F32, name="pA512", tag="pA512")
                nc.tensor.matmul(pqs[:], wqTs, xq[:])
                nc.scalar.copy(qs_sb[:], pqs[:])
                if DEBUG:
                    nc.sync.dma_start(dbg["dbg_qs"].ap(), qs_sb[:])


            # ============ phase D: CPB MLP (fp32r) ============
            with (
                tc.tile_pool(name="ps1", bufs=4, space="PSUM") as ps1,
                tc.tile_pool(name="ps2", bufs=2, space="PSUM") as ps2,
                tc.tile_pool(name="ps3", bufs=1, space="PSUM") as ps3,
            ):
                # two psum banks collect the transposed bias (one per j-half),
                # spilled to SBUF when full (after every 64 pairs)
                biasTp = [ps3.tile([128, 2 * NDS], F32, name=f"biasTp{i}", tag=f"biasTp{i}")
                          for i in range(2)]
                for it in range(4):
                    for pp in range(32):  # two queries... pair-iteration
                        kpair = it * 32 + pp
                        pre1 = ps1.tile([128, 2 * NDS], F32, name="pre1", tag="pre1")
                        h1 = h1p.tile([128, 2 * NDS], F32R, name="h1", tag="h1")
                        pre2 = ps2.tile([128, 2 * NDS], F32, name="pre2", tag="pre2")
                        h2 = h2p.tile([128, 2 * NDS], F32R, name="h2", tag="h2")
                        for half in range(2):
                            p = 2 * pp + half
                            sl = slice(half * NDS, (half + 1) * NDS)
                            a, m = p // 32, p % 32
                            nc.tensor.matmul(
                                pre1[:, sl],
                                maskr[64 * a:64 * (a + 1), 128 * m:128 * (m + 1)],
                                tT[it][64 * a:64 * (a + 1), :])
                        r1_act = kpair % 2 == 1
                        r2_act = kpair % 2 == 0
                        if r1_act:
                            nc.scalar.activation(h1[:], pre1[:], AF.Relu, bias=b1col)
                        else:
                            nc.vector.tensor_scalar(h1[:], pre1[:], b1col, 0.0, ALU.add, ALU.max)
                        for half in range(2):
                            sl = slice(half * NDS, (half + 1) * NDS)
                            nc.tensor.matmul(pre2[:, sl], w2bdr[:], h1[:, sl])
                        if r2_act:
                            nc.scalar.activation(h2[:], pre2[:], AF.Relu, bias=b2col)
                        else:
                            nc.vector.tensor_scalar(h2[:], pre2[:], b2col, 0.0, ALU.add, ALU.max)
                        for h1f in range(2):
                            for H in range(2):
                                outn = 2 * kpair + h1f
                                slot = outn % 128
                                nc.tensor.matmul(
                                    biasTp[H][:, 4 * slot:4 * slot + 4],
                                    h2[:, 256 * h1f + 128 * H:256 * h1f + 128 * H + 128],
                                    w3bdr[:])
                        if kpair % 64 == 63:
                            bank = kpair // 64
                            nc.vector.tensor_copy(
                                biasT_sb[:, QS * bank:QS * (bank + 1)],
                                biasTp[0][:])
                            nc.scalar.copy(
                                biasT_sb[:, QS * (2 + bank):QS * (2 + bank + 1)],
                                biasTp[1][:])
                            if bank == 0:
                                biasTp = [ps3.tile([128, 2 * NDS], F32,
                                                   name=f"biasTp{i}b", tag=f"biasTp{i}")
                                          for i in range(2)]

            if DEBUG:
                nc.sync.dma_start(dbg["dbg_bstk0"].ap(), biasT_sb[:, 0:NDS])
            # ============ phase E: attention ============
            with (
                tc.tile_pool(name="psE", bufs=2, space="PSUM") as psE,
                tc.tile_pool(name="psE1", bufs=1, space="PSUM") as psE1,
            ):
                # biasT_sb column decomposition:
                # col = 2048*H? no: region (2H+bank)*QS, inner 4*slot + 2c + o
                # with slot = (2*(32it+pp) + h1) % 128 and bank = itq = it//2.
                # As i_loc = 128it + 4pp + 2h1 + c runs over [128it, 128it+128),
                # (pp, h1, c) iterate with c innermost -- matching free order.
                bview = biasT_sb[:].rearrange(
                    "p (r itl pp h1 c o) -> p r itl pp h1 c o",
                    r=4, itl=2, pp=32, h1=2, c=2, o=2)

                for h in range(2):
                    expT = []
                    for H in range(2):
                        psim = psE.tile([128, QS], F32, name="psim", tag="psim")
                        nc.tensor.matmul(
                            psim[:], k_sb[32 * h:32 * (h + 1), H * 128:(H + 1) * 128],
                            qs_sb[32 * h:32 * (h + 1), :])
                        logit = wk.tile([128, QS], F32, name="logit", tag="logit")
                        for it in range(4):
                            itq, itl = it // 2, it % 2
                            nc.vector.scalar_tensor_tensor(
                                logit[:, 128 * it:128 * (it + 1)],
                                bview[:, 2 * H + itq, itl, :, :, :, h],
                                b3bc[:, h:h + 1],
                                psim[:, 128 * it:128 * (it + 1)],
                                ALU.add, ALU.add)
                        if DEBUG and h == 0 and H == 0:
                            nc.sync.dma_start(dbg["dbg_logit00"].ap(), logit[:])
                        et = wk.tile([128, QS], F32R, name="expT", tag="expT")
                        nc.scalar.activation(et[:], logit[:], AF.Exp)
                        expT.append(et)

                    # sums over j via ones-matmul, then reciprocal
                    psum_s = psE1.tile([1, QS], F32, name="psum_s", tag="psum_s")
                    for H in range(2):
                        nc.tensor.matmul(psum_s[:], ones_colr[:], expT[H][:],
                                         start=(H == 0), stop=(H == 1))
                    rs = rw.tile([1, QS], F32, name="rs", tag="rs")
                    nc.vector.reciprocal(rs[:], psum_s[:])
                    rsb = wk.tile([32, QS], F32, name="rsb", tag="rsb")
                    nc.gpsimd.partition_broadcast(rsb[:], rs[:])

                    pav = psE1.tile([32, QS], F32, name="pav", tag="pav")
                    for H in range(2):
                        nc.tensor.matmul(pav[:], vT[H][:, 32 * h:32 * (h + 1)], expT[H][:],
                                         start=(H == 0), stop=(H == 1))
                    nc.vector.tensor_tensor(avn[32 * h:32 * (h + 1), :], pav[:], rsb[:], ALU.mult)

                for m in range(2):
                    py = psE.tile([128, QS], F32, name="py", tag="py")
                    nc.tensor.matmul(py[:], woTr[:, m * 128:(m + 1) * 128], avn[:])
                    y_sb = wk.tile([128, QS], F32, name="y_sb", tag="y_sb")
                    nc.scalar.copy(y_sb[:], py[:])
                    nc.sync.dma_start(y_out.ap()[m * 128:(m + 1) * 128, :], y_sb[:])

    nc.compile()
    return nc


def _shard_inputs(inputs):
    """Build the 8 per-core input maps from the full inputs."""
    x = np.ascontiguousarray(inputs["x"][0])              # [256, 1024]
    wq, wk, wv = inputs["wq"], inputs["wk"], inputs["wv"]  # [4, 64, 64]
    wo = inputs["wo"]                                      # [256, 256]
    w_off_dw = inputs["w_off_dw"][:, 0, :]                 # [64, 6]
    b_off_dw = inputs["b_off_dw"]                          # [64]
    w_off_proj = inputs["w_off_proj"]                      # [64]
    w1 = inputs["cpb_w1"][:, 0]                            # [64]
    b1 = inputs["cpb_b1"]                                  # [64]
    w2 = inputs["cpb_w2"]                                  # [64, 64]
    b2 = inputs["cpb_b2"]                                  # [64]
    w3 = inputs["cpb_w3"]                                  # [2, 64]
    b3 = inputs["cpb_b3"]                                  # [2]

    f = np.float32
    w1sel = np.zeros((2, 128), f)
    w1sel[0, :64] = w1
    w1sel[1, 64:] = w1
    mask_st = np.zeros((128, 32 * 128), f)
    for band in range(2):
        for m in range(32):
            mask_st[64 * band + 2 * m:64 * band + 2 * m + 2, 128 * m:128 * (m + 1)] = w1sel
    b1col = np.concatenate([b1, b1]).astype(f)[:, None]
    w2bd = np.zeros((128, 128), f)
    w2bd[:64, :64] = w2.T
    w2bd[64:, 64:] = w2.T
    b2col = np.concatenate([b2, b2]).astype(f)[:, None]
    w3bd = np.zeros((128, 4), f)
    w3bd[:64, :2] = w3.T
    w3bd[64:, 2:] = w3.T
    b3bc = np.broadcast_to(b3.astype(f)[None, :], (128, 2)).copy()
    base_packed = np.zeros((128, 788), f)
    base_packed[:, 0:128] = w2bd
    base_packed[:, 128:256] = np.eye(128, dtype=f)
    base_packed[:, 776:777] = b1col
    base_packed[:, 777:778] = b2col
    base_packed[:, 778:780] = b3bc
    base_packed[:, 781:785] = w3bd

    in_maps = []
    for c in range(NCORES):
        g, qh = c // 2, c % 2
        xg = np.ascontiguousarray(x[64 * g:64 * (g + 1)], dtype=f)
        pk = base_packed.copy()
        pk[0:64, 256:320] = wq[g].T
        pk[0:64, 320:384] = wq[g].T * f(DH) ** f(-0.5)
        pk[0:64, 384:448] = wk[g].T
        pk[0:64, 448:512] = wv[g].T
        pk[0:64, 512:768] = wo[:, 64 * g:64 * (g + 1)].T
        pk[0:64, 768:774] = w_off_dw
        pk[0:64, 774] = b_off_dw
        pk[0:64, 775] = 0.5 * w_off_proj
        pk[:, 780] = f(QS * qh)
        m = {
            "xg": xg,
            "xq": np.ascontiguousarray(xg[:, QS * qh:QS * (qh + 1)]),
            "mask_st": mask_st,
            "packed": pk,
        }
        in_maps.append(m)
    return in_maps


def kernel(**inputs):
    if "nc" not in _CACHED:
        _CACHED["nc"] = build_nc()
    nc = _CACHED["nc"]
    in_maps = _shard_inputs(inputs)
    res = bass_utils.run_bass_kernel_spmd(nc, in_maps, core_ids=list(range(NCORES)))
    ys = [res.results[c]["y"] for c in range(NCORES)]
    bo = inputs["bo"]
    out = np.zeros((1, DIM, N), np.float32)
    for qh in range(2):
        acc = np.zeros((DIM, QS), np.float64)
        for g in range(G):
            acc += ys[2 * g + qh]
        out[0, :, QS * qh:QS * (qh + 1)] = (acc + bo.astype(np.float64)[:, None]).astype(np.float32)
    return out



# revision 17
# speedup vs baseline: 2.3013x; 2.3013x over previous
"""DeformableAttention1D on 8 TRN2 NeuronCores via Bass/Tile.

Sharding: core c handles offset-group g=c//2 (64 of 256 channels, 2 of 8 heads)
and query-half qh=c%2 (512 of 1024 positions). Each core computes its group's
offsets/gather/CPB/attention independently; the final output projection is
computed as a partial (wo sliced by group) and summed on the host (the
"all-reduce" of the output projection).

Key algorithmic choices vs a direct translation:
- The CPB relative-position-bias MLP has a SCALAR input t = sign(pos)*log1p(|pos|)
  with t bounded in [-1.12, 1.12] independent of the data, so the whole
  64-hidden-unit MLP is a fixed 1-D function R -> R^2. It is fit on the host
  (per-call, from the cpb weights only) with a degree-20 polynomial and
  evaluated on device as 4 Horner chains (fused mult-add DVE/Pool ops).
- The bias is evaluated on a 4x-coarse query grid (129 points per 512-query
  half) and linearly upsampled along the uniform query axis (fixed-stencil
  blend, no gather). Measured end-to-end rel err ~5e-3 (gate 2e-2).
- The kv grid_sample gather uses the hat-function identity
  S[n,j] = relu(1 - |n - p_j|), which reproduces the reference's masked
  bilinear weights exactly, and the band structure p_j ~= 4.016*j +- 17 so
  each 128-column j-half only needs 5 of the 8 n-tiles.
- All matmul operands are fp32r (1 cycle/column); bias is added to the
  sim PSUM via an identity-matmul accumulate, so ACT's Exp reads PSUM
  directly.

The ACT engine is restricted to ONE table set (natural_log_exp_and_others:
Exp/Ln/Relu/Copy/Identity/Square) because runtime table swaps are broken in
this environment; tanh and erf(gelu) are composed from Exp + DVE ops.
"""
import os
import sys

sys.path.insert(0, "/opt/trn_rl_repo")

import numpy as np

import concourse.bacc as bacc
import concourse.bass as bass
import concourse.mybir as mybir
import concourse.tile as tile
import concourse.bass_utils as bass_utils

F32 = mybir.dt.float32
F32R = mybir.dt.float32r
I32 = mybir.dt.int32
U32 = mybir.dt.uint32
AF = mybir.ActivationFunctionType
ALU = mybir.AluOpType

# model dims (hardcoded per problem spec)
DIM = 256
N = 1024
G = 4
DH = 32
NDS = 256          # downsampled kv positions
QS = 512           # queries per core
DPG = 64           # channels per group
OFF_K = 6
DS = 4             # downsample stride
OFF_SCALE = 4.0
NCORES = 8

DEG = 16           # CPB polynomial degree
NCG = 129          # coarse query-grid points (4x downsample of 512, + endpoint)

# A&S 7.1.26 3-term erf coefficients (|err| <= 2.5e-5)
ERF3_P = 0.47047
ERF3_A = (0.3480242, -0.0958798, 0.7478556)

# packed_r (fp32r matmul stationaries) column layout
PR_WQT = 0
PR_WKT = 64
PR_WVT = 128
PR_WOT = 192       # [64, 256]
PR_WPROJ = 448     # [64, 1]
PR_EYE = 449       # [128, 128]
PR_W = 577

# packed_f (DVE scalar-pointer columns) layout
PF_WDW = 0         # [64, 6]
PF_BODW = 6
PF_POLY = 7        # [128, 42]: c0..c20 for o=0, then o=1
PF_QB = 49         # qh * 1024/1023
PF_W = 50

_CACHED = {}


def _patch_act_tables():
    """Restrict activation-table selection to the single set that covers all
    ACT functions used by this kernel, so exactly one table load is emitted
    (runtime table swaps do not work in this environment)."""
    import concourse.hw_specs as hw_specs

    if getattr(bacc, "_deform_act_patch", False):
        return
    orig = hw_specs.get_activation_tables

    keep = "natural_log_exp_and_others"

    def patched(module_arch):
        tabs = orig(module_arch)
        keep_funcs = tabs[keep]
        out = {}
        for name, funcs in tabs.items():
            if name == keep:
                out[name] = funcs
            else:
                out[name] = funcs - keep_funcs
        return out

    bacc.get_activation_tables = patched
    bacc._deform_act_patch = True


def _erf_gelu(nc, sb, out_ap, x_ap, shape):
    """out = x * (1 + erf(x/sqrt(2)))  (the 0.5 is folded into wproj)."""
    P, Nf = shape
    sq = sb.tile([P, Nf], F32, name="gelu_sq", tag="gelu_sq")
    nc.scalar.activation(sq[:], x_ap, AF.Square)
    e = sb.tile([P, Nf], F32, name="gelu_e", tag="gelu_e")
    nc.scalar.activation(e[:], sq[:], AF.Exp, scale=-0.5)   # exp(-x^2/2)
    ax = sb.tile([P, Nf], F32, name="gelu_ax", tag="gelu_ax")
    nc.vector.scalar_tensor_tensor(ax[:], x_ap, -1.0, x_ap, ALU.mult, ALU.max)
    den = sb.tile([P, Nf], F32, name="gelu_den", tag="gelu_den")
    nc.vector.tensor_scalar(den[:], ax[:], float(ERF3_P / np.sqrt(2.0)), 1.0,
                            ALU.mult, ALU.add)
    r = sb.tile([P, Nf], F32, name="gelu_r", tag="gelu_r")
    nc.vector.reciprocal(r[:], den[:])
    # poly = a1 r + a2 r^2 + a3 r^3
    p1 = sb.tile([P, Nf], F32, name="gelu_p1", tag="gelu_p1")
    nc.vector.tensor_scalar(p1[:], r[:], ERF3_A[2], ERF3_A[1], ALU.mult, ALU.add)
    u = sb.tile([P, Nf], F32, name="gelu_u", tag="gelu_u")
    nc.vector.tensor_tensor(u[:], p1[:], r[:], ALU.mult)
    poly = sb.tile([P, Nf], F32, name="gelu_poly", tag="gelu_poly")
    nc.vector.scalar_tensor_tensor(poly[:], u[:], ERF3_A[0], r[:], ALU.add, ALU.mult)
    # erfa = 1 - poly*e ; erf = sign(x)*erfa ; out = (1+erf)*x
    q_ = sb.tile([P, Nf], F32, name="gelu_q", tag="gelu_q")
    nc.vector.tensor_tensor(q_[:], poly[:], e[:], ALU.mult)
    erfa = sb.tile([P, Nf], F32, name="gelu_erfa", tag="gelu_erfa")
    nc.vector.tensor_scalar(erfa[:], q_[:], -1.0, 1.0, ALU.mult, ALU.add)
    sgn = sb.tile([P, Nf], U32, name="gelu_sgn", tag="gelu_sgn")
    nc.vector.tensor_scalar(sgn[:], x_ap.bitcast(U32), 0x80000000, None,
                            ALU.bitwise_and)
    erfs = sb.tile([P, Nf], F32, name="gelu_erfs", tag="gelu_erfs")
    nc.vector.tensor_tensor(erfs[:].bitcast(U32), erfa[:].bitcast(U32), sgn[:],
                            ALU.bitwise_or)
    nc.vector.tensor_scalar(erfs[:], erfs[:], 1.0, None, ALU.add)
    nc.vector.tensor_tensor(out_ap, erfs[:], x_ap, ALU.mult)


def _tanh_rows(nc, sb, out_ap, x_ap, shape):
    """out = tanh(x) = sign(x) * (1 - 2/(exp(2*min(|x|,30))+1)) on small tiles."""
    P, Nf = shape
    ax = sb.tile([P, Nf], F32, name="th_ax", tag="th_ax")
    nc.vector.scalar_tensor_tensor(ax[:], x_ap, -1.0, x_ap, ALU.mult, ALU.max)
    nc.vector.tensor_scalar(ax[:], ax[:], 30.0, None, ALU.min)
    e = sb.tile([P, Nf], F32, name="th_e", tag="th_e")
    nc.scalar.activation(e[:], ax[:], AF.Exp, scale=2.0)
    nc.vector.tensor_scalar(e[:], e[:], 1.0, None, ALU.add)
    r = sb.tile([P, Nf], F32, name="th_r", tag="th_r")
    nc.vector.reciprocal(r[:], e[:])
    nc.vector.tensor_scalar(r[:], r[:], -2.0, 1.0, ALU.mult, ALU.add)
    sgn = sb.tile([P, Nf], U32, name="th_sgn", tag="th_sgn")
    nc.vector.tensor_scalar(sgn[:], x_ap.bitcast(U32), 0x80000000, None,
                            ALU.bitwise_and)
    nc.vector.tensor_tensor(out_ap.bitcast(U32), r[:].bitcast(U32), sgn[:],
                            ALU.bitwise_or)


def build_nc():
    _patch_act_tables()
    nc = bacc.Bacc("TRN2", target_bir_lowering=False, debug=False,
                   num_devices=NCORES)

    din = {}
    din["xg"] = nc.dram_tensor("xg", [DPG, N], F32R, kind="ExternalInput")
    din["xq"] = nc.dram_tensor("xq", [DPG, QS], F32R, kind="ExternalInput")
    din["xgT"] = nc.dram_tensor("xgT", [128, 8 * DPG], F32R, kind="ExternalInput")
    din["packed_r"] = nc.dram_tensor("packed_r", [128, PR_W], F32R,
                                     kind="ExternalInput")
    din["packed_f"] = nc.dram_tensor("packed_f", [128, PF_W], F32,
                                     kind="ExternalInput")
    y_out = nc.dram_tensor("y", [DIM, QS], F32, kind="ExternalOutput")

    with tile.TileContext(nc) as tc:
        with (
            tc.tile_pool(name="const", bufs=1) as cst,
            tc.tile_pool(name="work", bufs=2) as wk,
            tc.tile_pool(name="rows", bufs=1) as rw,
            tc.tile_pool(name="persist", bufs=1) as pe_pool,
        ):
            # ---- input DMAs ----
            xg = cst.tile([DPG, N], F32R, name="xg", tag="xg")
            nc.sync.dma_start(xg[:], din["xg"].ap())
            xq = cst.tile([DPG, QS], F32R, name="xq", tag="xq")
            nc.sync.dma_start(xq[:], din["xq"].ap())
            xgT = cst.tile([128, 8 * DPG], F32R, name="xgT", tag="xgT")
            nc.sync.dma_start(xgT[:], din["xgT"].ap())
            pr = cst.tile([128, PR_W], F32R, name="pr", tag="pr")
            nc.sync.dma_start(pr[:], din["packed_r"].ap())
            pf = cst.tile([128, PF_W], F32, name="pf", tag="pf")
            nc.sync.dma_start(pf[:], din["packed_f"].ap())

            wqT = pr[0:DPG, PR_WQT:PR_WQT + 64]
            wkT = pr[0:DPG, PR_WKT:PR_WKT + 64]
            wvT = pr[0:DPG, PR_WVT:PR_WVT + 64]
            woT = pr[0:DPG, PR_WOT:PR_WOT + 256]
            wproj = pr[0:DPG, PR_WPROJ:PR_WPROJ + 1]
            eyer = pr[:, PR_EYE:PR_EYE + 128]
            wdw = pf[0:DPG, PF_WDW:PF_WDW + OFF_K]
            bodw = pf[0:DPG, PF_BODW:PF_BODW + 1]
            qbcol = pf[:, PF_QB:PF_QB + 1]

            def pcol(o, k):
                return pf[:, PF_POLY + 21 * o + k:PF_POLY + 21 * o + k + 1]

            ones_col = cst.tile([128, 1], F32, name="ones", tag="ones")
            nc.gpsimd.memset(ones_col[:], 1.0)
            # dummy activation: triggers the (single) ACT table load at t=0 so
            # it overlaps the input DMAs
            warm = cst.tile([128, 1], F32, name="warm", tag="warm")
            nc.scalar.activation(warm[:], ones_col[:], AF.Relu)
            ones_colr = cst.tile([128, 1], F32R, name="onesr", tag="onesr")
            nc.vector.tensor_copy(ones_colr[:], ones_col[:])
            icol = cst.tile([128, 1], I32, name="icol", tag="icol")
            nc.gpsimd.iota(icol[:], pattern=[[0, 1]], base=0, channel_multiplier=1)
            icolf = cst.tile([128, 1], F32, name="icolf", tag="icolf")
            nc.vector.tensor_copy(icolf[:], icol[:])

            # persistent SBUF tiles crossing phase boundaries
            q_pad = pe_pool.tile([DPG, N + 2], F32, name="q_pad", tag="q_pad")
            kv_sb = pe_pool.tile([DPG, NDS], F32R, name="kv_sb", tag="kv_sb")
            k_sb = pe_pool.tile([DPG, NDS], F32R, name="k_sb", tag="k_sb")
            v_sb = pe_pool.tile([DPG, NDS], F32R, name="v_sb", tag="v_sb")
            vT = [pe_pool.tile([128, DPG], F32R, name=f"vT{H}", tag=f"vT{H}")
                  for H in range(2)]
            qs_sb = pe_pool.tile([DPG, QS], F32R, name="qs_sb", tag="qs_sb")
            t_H = [pe_pool.tile([128, NCG], F32, name=f"t{H}", tag=f"t{H}")
                   for H in range(2)]
            bias_f = {}
            for H in range(2):
                for o in range(2):
                    bias_f[(H, o)] = pe_pool.tile(
                        [128, QS], F32R, name=f"bias{H}{o}", tag=f"bias{H}{o}")
            avn = pe_pool.tile([DPG, QS], F32R, name="avn", tag="avn")
            bc_p = pe_pool.tile([128, NDS], F32, name="bc_p", tag="bc_p")
            vcols = pe_pool.tile([128, 2], F32, name="vcols", tag="vcols")

            # ============ phase A: q + offsets ============
            with (
                tc.tile_pool(name="psQ", bufs=2, space="PSUM") as psQ,
                tc.tile_pool(name="psT", bufs=2, space="PSUM") as psT,
                tc.tile_pool(name="psS", bufs=2, space="PSUM") as psS,
                tc.tile_pool(name="psR", bufs=1, space="PSUM") as psR,
            ):
                nc.gpsimd.memset(q_pad[:], 0.0)
                for h in range(2):
                    pq = psQ.tile([DPG, QS], F32, name="pq", tag="pq")
                    nc.tensor.matmul(pq[:], wqT, xg[:, h * QS:(h + 1) * QS])
                    nc.scalar.copy(q_pad[:, 1 + h * QS:1 + (h + 1) * QS], pq[:])

                # depthwise strided conv (6 taps)
                acc = wk.tile([DPG, NDS], F32, name="conv_acc", tag="conv_acc")
                nc.vector.tensor_scalar(
                    acc[:], q_pad[:, 0:N - 3:DS], wdw[:, 0:1], bodw,
                    ALU.mult, ALU.add)
                for kk in range(1, OFF_K):
                    nc.vector.scalar_tensor_tensor(
                        acc[:], q_pad[:, kk:kk + N - 3:DS], wdw[:, kk:kk + 1],
                        acc[:], ALU.mult, ALU.add)

                gl = wk.tile([DPG, NDS], F32R, name="gelu_out", tag="gelu_out")
                _erf_gelu(nc, wk, gl[:], acc[:], [DPG, NDS])

                pproj = psR.tile([1, NDS], F32, name="pproj", tag="pproj")
                nc.tensor.matmul(pproj[:], wproj, gl[:])
                proj_sb = rw.tile([1, NDS], F32, name="proj_sb", tag="proj_sb")
                nc.vector.tensor_copy(proj_sb[:], pproj[:])
                th = rw.tile([1, NDS], F32, name="th", tag="th")
                _tanh_rows(nc, rw, th[:], proj_sb[:], [1, NDS])

                # vgrid = j + 4*tanh ; vp = vgrid*2/255 ; p = vp*512 - 0.5
                iotaj = rw.tile([1, NDS], I32, name="iotaj", tag="iotaj")
                nc.gpsimd.iota(iotaj[:], pattern=[[1, NDS]], base=0,
                               channel_multiplier=0)
                iotajf = rw.tile([1, NDS], F32, name="iotajf", tag="iotajf")
                nc.vector.tensor_copy(iotajf[:], iotaj[:])
                vgrid = rw.tile([1, NDS], F32, name="vgrid", tag="vgrid")
                nc.vector.scalar_tensor_tensor(vgrid[:], th[:], OFF_SCALE,
                                               iotajf[:], ALU.mult, ALU.add)
                vp = rw.tile([1, NDS], F32, name="vp", tag="vp")
                nc.vector.tensor_scalar(vp[:], vgrid[:], float(2.0 / (NDS - 1)),
                                        None, ALU.mult)
                p_row = rw.tile([1, NDS], F32, name="p_row", tag="p_row")
                nc.vector.tensor_scalar(p_row[:], vp[:], float(N / 2.0), -0.5,
                                        ALU.mult, ALU.add)

                # vp as per-partition columns (PE transpose of the row)
                for H in range(2):
                    ptv = psT.tile([128, 128], F32, name="ptv", tag="ptv")
                    nc.tensor.transpose(ptv[:, 0:1],
                                        vp[:, H * 128:(H + 1) * 128],
                                        ones_col[0:1, 0:1])
                    nc.vector.tensor_copy(vcols[:, H:H + 1], ptv[:, 0:1])
                # p broadcast across partitions for the hat build
                nc.gpsimd.partition_broadcast(bc_p[:], p_row[:])

                # ---- phase C: coarse t grid (both j-halves) ----
                iog = rw.tile([128, NCG], I32, name="iog", tag="iog")
                nc.gpsimd.iota(iog[:], pattern=[[1, NCG]], base=0,
                               channel_multiplier=0)
                gqc = rw.tile([128, NCG], F32, name="gqc", tag="gqc")
                nc.vector.tensor_copy(gqc[:], iog[:])
                nc.gpsimd.tensor_scalar(gqc[:], gqc[:], float(8.0 / (N - 1)),
                                        qbcol, ALU.mult, ALU.add)
                for H in range(2):
                    pos = wk.tile([128, NCG], F32, name="pos", tag="pos")
                    nc.gpsimd.tensor_scalar(pos[:], gqc[:], vcols[:, H:H + 1],
                                            None, ALU.subtract)
                    apos = wk.tile([128, NCG], F32, name="apos", tag="apos")
                    nc.vector.scalar_tensor_tensor(apos[:], pos[:], -1.0, pos[:],
                                                   ALU.mult, ALU.max)
                    aln = wk.tile([128, NCG], F32, name="aln", tag="aln")
                    nc.scalar.activation(aln[:], apos[:], AF.Ln, bias=1.0)
                    sgn = wk.tile([128, NCG], U32, name="psgn", tag="psgn")
                    nc.vector.tensor_scalar(sgn[:], pos[:].bitcast(U32),
                                            0x80000000, None, ALU.bitwise_and)
                    nc.vector.tensor_tensor(t_H[H][:].bitcast(U32),
                                            aln[:].bitcast(U32), sgn[:],
                                            ALU.bitwise_or)

                # ---- phase D: CPB polynomial (Horner, DVE; Pool lacks stt) ----
                bc_coarse = {}
                for H, o in ((0, 0), (0, 1), (1, 0), (1, 1)):
                    t_ap = t_H[H][:]
                    u = pe_pool.tile([128, NCG], F32, name=f"hu{H}{o}",
                                     tag=f"hu{H}{o}")
                    nc.vector.tensor_scalar(u[:], t_ap, pcol(o, DEG), None,
                                            ALU.mult)
                    for k in range(DEG - 1, 0, -1):
                        nc.vector.scalar_tensor_tensor(
                            u[:], u[:], pcol(o, k), t_ap, ALU.add, ALU.mult)
                    b = pe_pool.tile([128, NCG], F32, name=f"hb{H}{o}",
                                     tag=f"hb{H}{o}")
                    nc.vector.tensor_scalar(b[:], u[:], pcol(o, 0), None,
                                            ALU.add)
                    bc_coarse[(H, o)] = b

                # upsample 4x along the query axis into fp32r bias tiles
                for H in range(2):
                    for o in range(2):
                        b = bc_coarse[(H, o)]
                        db = wk.tile([128, 128], F32, name="db", tag="db")
                        nc.vector.tensor_tensor(db[:], b[:, 1:NCG],
                                                b[:, 0:NCG - 1], ALU.subtract)
                        bf = bias_f[(H, o)]
                        nc.scalar.copy(bf[:, 0:QS:4], b[:, 0:128])
                        for r in (1, 2, 3):
                            nc.vector.scalar_tensor_tensor(
                                bf[:, r:QS:4], db[:], float(r / 4.0),
                                b[:, 0:128], ALU.mult, ALU.add)

                # ---- phase B: banded hat gather (on Pool), kv, k, v, qs ----
                # S[n,j] = relu(1-|n-p_j|) = min(relu(1-t1), relu(1+t1)),
                # t1 = p_j - n; only ts/tt ops (Pool has no stt)
                pkv = [None, None]
                for H in range(2):
                    tiles = (0, 1, 2, 3, 4) if H == 0 else (3, 4, 5, 6, 7)
                    pkv[H] = psS.tile([DPG, 128], F32, name="pkv", tag="pkv")
                    for ii, tn in enumerate(tiles):
                        t1 = wk.tile([128, 128], F32, name="hat1", tag="hat1")
                        S = wk.tile([128, 128], F32R, name="hatS", tag="hatS")
                        if H == 0:
                            # DVE: |t1| via stt, then 1-|t1|, relu
                            nc.vector.tensor_scalar(
                                t1[:], bc_p[:, 0:128], icolf[:],
                                float(128 * tn), ALU.subtract, ALU.subtract)
                            a = wk.tile([128, 128], F32, name="hat2", tag="hat2")
                            nc.vector.scalar_tensor_tensor(
                                a[:], t1[:], -1.0, t1[:], ALU.mult, ALU.max)
                            m = wk.tile([128, 128], F32, name="hat3", tag="hat3")
                            nc.vector.tensor_scalar(m[:], a[:], -1.0, 1.0,
                                                    ALU.mult, ALU.add)
                            nc.vector.tensor_scalar(S[:], m[:], 0.0, None,
                                                    ALU.max)
                        else:
                            # Pool (no stt/min): relu(relu(1-t1)-relu(-2*t1))
                            nc.gpsimd.tensor_scalar(
                                t1[:], bc_p[:, 128:256], icolf[:],
                                float(128 * tn), ALU.subtract, ALU.subtract)
                            r1 = wk.tile([128, 128], F32, name="hat2", tag="hat2")
                            nc.gpsimd.tensor_scalar(r1[:], t1[:], -1.0, 1.0,
                                                    ALU.mult, ALU.add)
                            nc.gpsimd.tensor_scalar(r1[:], r1[:], 0.0, None,
                                                    ALU.max)
                            r2 = wk.tile([128, 128], F32, name="hat3", tag="hat3")
                            nc.gpsimd.tensor_scalar(r2[:], t1[:], -2.0, 0.0,
                                                    ALU.mult, ALU.max)
                            d = wk.tile([128, 128], F32, name="hat4", tag="hat4")
                            nc.gpsimd.tensor_tensor(d[:], r1[:], r2[:],
                                                    ALU.subtract)
                            nc.gpsimd.tensor_scalar(S[:], d[:], 0.0, None,
                                                    ALU.max)
                        nc.tensor.matmul(pkv[H][:],
                                         xgT[:, DPG * tn:DPG * (tn + 1)], S[:],
                                         start=(ii == 0), stop=(ii == 4))
                    nc.scalar.copy(kv_sb[:, H * 128:(H + 1) * 128], pkv[H][:])

                pk = psS.tile([DPG, NDS], F32, name="pko", tag="pko", bufs=1)
                nc.tensor.matmul(pk[:], wkT, kv_sb[:])
                nc.scalar.copy(k_sb[:], pk[:])
                pv = psS.tile([DPG, NDS], F32, name="pko", tag="pko", bufs=1)
                nc.tensor.matmul(pv[:], wvT, kv_sb[:])
                nc.scalar.copy(v_sb[:], pv[:])
                for H in range(2):
                    pt = psT.tile([128, 128], F32R, name="ptv", tag="ptv")
                    nc.tensor.transpose(pt[:, 0:DPG],
                                        v_sb[:, H * 128:(H + 1) * 128],
                                        eyer[0:DPG, 0:DPG])
                    nc.scalar.copy(vT[H][:], pt[:, 0:DPG])

                # scaled q for this core's query half
                pqs = psQ.tile([DPG, QS], F32, name="pq", tag="pq")
                nc.tensor.matmul(pqs[:], wqT, xq[:])
                nc.vector.tensor_scalar(qs_sb[:], pqs[:],
                                        float(DH ** -0.5), None, ALU.mult)

            # ============ phase E: attention ============
            with (
                tc.tile_pool(name="psE", bufs=2, space="PSUM") as psE,
                tc.tile_pool(name="psE1", bufs=2, space="PSUM") as psE1,
            ):
                for h in range(2):
                    expT = []
                    for H in range(2):
                        psim = psE.tile([128, QS], F32, name="psim", tag="psim")
                        nc.tensor.matmul(
                            psim[:], k_sb[32 * h:32 * (h + 1),
                                          H * 128:(H + 1) * 128],
                            qs_sb[32 * h:32 * (h + 1), :],
                            start=True, stop=False)
                        nc.tensor.matmul(psim[:], eyer, bias_f[(H, h)][:],
                                         start=False, stop=True)
                        et = wk.tile([128, QS], F32R, name="expT", tag="expT")
                        nc.scalar.activation(et[:], psim[:], AF.Exp)
                        expT.append(et)

                    psum_s = psE1.tile([1, QS], F32, name="psum_s", tag="psum_s")
                    for H in range(2):
                        nc.tensor.matmul(psum_s[:], ones_colr[:], expT[H][:],
                                         start=(H == 0), stop=(H == 1))
                    rs = rw.tile([1, QS], F32, name="rs", tag="rs")
                    nc.vector.reciprocal(rs[:], psum_s[:])
                    rsb = wk.tile([32, QS], F32, name="rsb", tag="rsb")
                    nc.gpsimd.partition_broadcast(rsb[:], rs[:])

                    pav = psE1.tile([32, QS], F32, name="pav", tag="pav")
                    for H in range(2):
                        nc.tensor.matmul(pav[:], vT[H][:, 32 * h:32 * (h + 1)],
                                         expT[H][:], start=(H == 0),
                                         stop=(H == 1))
                    nc.vector.tensor_tensor(avn[32 * h:32 * (h + 1), :], pav[:],
                                            rsb[:], ALU.mult)

                for m in range(2):
                    py = psE.tile([128, QS], F32, name="py", tag="py")
                    nc.tensor.matmul(py[:], woT[:, m * 128:(m + 1) * 128],
                                     avn[:])
                    y_sb = wk.tile([128, QS], F32, name="y_sb", tag="y_sb")
                    nc.scalar.copy(y_sb[:], py[:])
                    nc.sync.dma_start(y_out.ap()[m * 128:(m + 1) * 128, :],
                                      y_sb[:])

    nc.compile()
    return nc


def _fit_poly(w1, b1, w2, b2, w3, b3):
    """Least-squares Chebyshev fit of the scalar CPB MLP over the fixed
    t-domain; returns [128, 42] of broadcast power-basis coefficients
    (ascending, b3 folded into c0)."""
    tg = np.linspace(-1.12, 1.12, 16001)
    h1 = np.maximum(tg[:, None] * w1.astype(np.float64) + b1.astype(np.float64), 0)
    h2 = np.maximum(h1 @ w2.astype(np.float64).T + b2.astype(np.float64), 0)
    fg = h2 @ w3.astype(np.float64).T                       # [T, 2]
    cols = np.zeros((128, 42), np.float32)
    for o in range(2):
        ch = np.polynomial.chebyshev.Chebyshev.fit(tg, fg[:, o], DEG)
        pc = np.polynomial.chebyshev.cheb2poly(ch.convert().coef)
        pc = np.pad(pc, (0, DEG + 1 - len(pc)))
        pc[0] += float(b3[o])
        cols[:, 21 * o:21 * o + DEG + 1] = pc.astype(np.float32)[None, :]
    return cols


def _shard_inputs(inputs):
    """Build the 8 per-core input maps from the full inputs."""
    x = np.ascontiguousarray(inputs["x"][0])               # [256, 1024]
    wq, wk, wv = inputs["wq"], inputs["wk"], inputs["wv"]  # [4, 64, 64]
    wo = inputs["wo"]                                      # [256, 256]
    w_off_dw = inputs["w_off_dw"][:, 0, :]                 # [64, 6]
    b_off_dw = inputs["b_off_dw"]                          # [64]
    w_off_proj = inputs["w_off_proj"]                      # [64]
    f = np.float32
    polyc = _fit_poly(inputs["cpb_w1"][:, 0], inputs["cpb_b1"],
                      inputs["cpb_w2"], inputs["cpb_b2"],
                      inputs["cpb_w3"], inputs["cpb_b3"])

    base_r = np.zeros((128, PR_W), f)
    base_r[:, PR_EYE:PR_EYE + 128] = np.eye(128, dtype=f)
    base_f = np.zeros((128, PF_W), f)
    base_f[:, PF_POLY:PF_POLY + 42] = polyc

    in_maps = []
    for c in range(NCORES):
        g, qh = c // 2, c % 2
        xg = np.ascontiguousarray(x[64 * g:64 * (g + 1)], dtype=f)
        xgT = np.zeros((128, 8 * DPG), f)
        for t in range(8):
            xgT[:, DPG * t:DPG * (t + 1)] = xg[:, 128 * t:128 * (t + 1)].T
        pr = base_r.copy()
        pr[0:DPG, PR_WQT:PR_WQT + 64] = wq[g].T
        pr[0:DPG, PR_WKT:PR_WKT + 64] = wk[g].T
        pr[0:DPG, PR_WVT:PR_WVT + 64] = wv[g].T
        pr[0:DPG, PR_WOT:PR_WOT + 256] = wo[:, 64 * g:64 * (g + 1)].T
        pr[0:DPG, PR_WPROJ] = 0.5 * w_off_proj
        pfc = base_f.copy()
        pfc[0:DPG, PF_WDW:PF_WDW + OFF_K] = w_off_dw
        pfc[0:DPG, PF_BODW] = b_off_dw
        pfc[:, PF_QB] = f(qh * N / (N - 1.0))
        in_maps.append({"xg": xg,
                        "xq": np.ascontiguousarray(xg[:, QS * qh:QS * (qh + 1)]),
                        "xgT": xgT, "packed_r": pr, "packed_f": pfc})
    return in_maps


def kernel(**inputs):
    if "nc" not in _CACHED:
        _CACHED["nc"] = build_nc()
    nc = _CACHED["nc"]
    in_maps = _shard_inputs(inputs)
    res = bass_utils.run_bass_kernel_spmd(nc, in_maps, core_ids=list(range(NCORES)))
    ys = [res.results[c]["y"] for c in range(NCORES)]
    bo = inputs["bo"]
    out = np.zeros((1, DIM, N), np.float32)
    for qh in range(2):
        acc = np.zeros((DIM, QS), np.float64)
        for g in range(G):
            acc += ys[2 * g + qh]
        out[0, :, QS * qh:QS * (qh + 1)] = (
            acc + bo.astype(np.float64)[:, None]).astype(np.float32)
    return out
